# revision 1
# baseline (speedup 1.0000x reference)
"""Trainium2 Bass kernel for nn_BoundleAdjustment (2M observations).

Two launches on all 8 NeuronCores (observations data-parallel, M/8 per core):

Launch A (device): converts the 4096-row pose table (translation+quaternion)
into per-pose rotation matrices R = f(q/|q|) on the Vector engine
([128, 32] planar layout, one reciprocal for the 2/|q|^2 scale).

Host staging (indexing only): gathers the derived R table, raw pose
translations, and patch rows by poses_idx/patch_idx, and lays per-observation
records out as 19 planar [128, 2048] f32 streams per core.

Launch B (device): streams the planes through SBUF in 2 chunks and computes
r = R*pts + t, cart->polar (Square/Sqrt/Arctan/Sign on the Scalar engine,
atan2 quadrant fixup, all divisions via one merged Vector-engine reciprocal
using 1/a = b/(a*b)), and the weighted residual.  DMA issue lives on the SP
queue so the Vector engine stays on math; ~35 vector ops per observation.
"""

import numpy as np

M = 2097152
NCORES = 8
N = M // NCORES
P = 128
COLS = N // P            # 2048
CC = 1024
NCH = COLS // CC
NPOSE = 4096
PC = NPOSE // P          # 32 cols for pose table
PI = float(np.pi)

# launch B planes (ordered so the rot-x dependency chain's planes land first):
# R00 R01 R02 px py pz tx | R10 R11 R12 ty | R20 R21 R22 tz | X Y Z w
NPB = 19

_CACHE = {}


def _build_posetab():
    import concourse.bass as bass
    import concourse.tile as tile
    from concourse import bacc, mybir

    nc = bacc.Bacc("TRN2", target_bir_lowering=False, debug=False,
                   num_devices=NCORES)
    f32 = mybir.dt.float32
    OP = mybir.AluOpType
    q_d = nc.declare_dram_parameter("q", [4, P, PC], f32, isOutput=False)
    r_d = nc.declare_dram_parameter("rtab", [9, P, PC], f32, isOutput=True)

    with tile.TileContext(nc) as tc:
        with tc.tile_pool(name="pp", bufs=40) as pp:
            cnt = [0]

            def T():
                cnt[0] += 1
                return pp.tile([P, PC], f32, tag="t", name=f"pt{cnt[0]}")

            vec = nc.vector

            def tt(a, b, op):
                d = T()
                vec.tensor_tensor(out=d[:], in0=a[:], in1=b[:], op=op)
                return d

            qs = []
            for k in range(4):
                t = pp.tile([P, PC], f32, tag="t", name=f"qin{k}")
                nc.sync.dma_start(t[:], q_d[k])
                qs.append(t)
            qx, qy, qz, qw = qs

            xx = tt(qx, qx, OP.mult); yy = tt(qy, qy, OP.mult)
            zz = tt(qz, qz, OP.mult); ww = tt(qw, qw, OP.mult)
            xy = tt(qx, qy, OP.mult); xz = tt(qx, qz, OP.mult)
            yz = tt(qy, qz, OP.mult)
            wx = tt(qw, qx, OP.mult); wy = tt(qw, qy, OP.mult)
            wz = tt(qw, qz, OP.mult)

            s01 = tt(xx, yy, OP.add)
            s23 = tt(zz, ww, OP.add)
            s = tt(s01, s23, OP.add)
            d1 = T()
            vec.tensor_scalar(out=d1[:], in0=s[:], scalar1=0.5, scalar2=None,
                              op0=OP.mult)
            u = T()
            vec.reciprocal(u[:], d1[:])        # u = 2/|q|^2

            outs = {}

            def diag(m1, m2, nm):
                a = tt(m1, m2, OP.add)
                b = tt(a, u, OP.mult)
                d = T()
                vec.tensor_scalar(out=d[:], in0=b[:], scalar1=-1.0,
                                  scalar2=1.0, op0=OP.mult, op1=OP.add)
                outs[nm] = d

            def offd(m1, m2, op, nm):
                a = tt(m1, m2, op)
                outs[nm] = tt(a, u, OP.mult)

            diag(yy, zz, "R00"); diag(xx, zz, "R11"); diag(xx, yy, "R22")
            offd(xy, wz, OP.subtract, "R01"); offd(xz, wy, OP.add, "R02")
            offd(xy, wz, OP.add, "R10"); offd(yz, wx, OP.subtract, "R12")
            offd(xz, wy, OP.subtract, "R20"); offd(yz, wx, OP.add, "R21")

            for i, nm in enumerate(("R00", "R01", "R02", "R10", "R11", "R12",
                                    "R20", "R21", "R22")):
                nc.sync.dma_start(r_d[i], outs[nm][:])
    nc.finalize()
    return nc


def _build_main():
    import concourse.bass as bass
    import concourse.tile as tile
    from concourse import bacc, mybir

    nc = bacc.Bacc("TRN2", target_bir_lowering=False, debug=False,
                   num_devices=NCORES)
    f32 = mybir.dt.float32
    AF = mybir.ActivationFunctionType
    OP = mybir.AluOpType
    in_d = nc.declare_dram_parameter("in", [NPB, P, COLS], f32, isOutput=False)
    out_d = nc.declare_dram_parameter("out", [3, P, COLS], f32, isOutput=True)

    with tile.TileContext(nc) as tc:
        with tc.tile_pool(name="inp", bufs=2) as inp, \
             tc.tile_pool(name="tmpp", bufs=17) as tmpp:
            chunks = [(0, 512), (512, 1024), (1536, 512)]
            for ch, (off, cc) in enumerate(chunks):
                ins = []
                for k in range(NPB):
                    # target/weight planes (15-18) are consumed at chunk
                    # end; single buffers suffice and free SBUF for tmpp
                    t = inp.tile([P, cc], f32, tag=f"in{k}", name=f"in{k}_{ch}",
                                 bufs=1 if k >= 15 else 2)
                    nc.sync.dma_start(t[:], in_d[k, :, off:off + cc])
                    ins.append(t)
                (R00, R01, R02, px, py, pz, tx, R10, R11, R12, ty,
                 R20, R21, R22, tz, X, Y, Z, W) = ins

                cnt = [0]

                def T():
                    cnt[0] += 1
                    return tmpp.tile([P, cc], f32, tag="tmp",
                                     name=f"tmp{ch}_{cnt[0]}")

                vec, act = nc.vector, nc.scalar

                def tt(a, b, op):
                    d = T()
                    vec.tensor_tensor(out=d[:], in0=a[:], in1=b[:], op=op)
                    return d

                def sq(a):
                    d = T()
                    act.activation(d[:], a[:], AF.Square)
                    return d

                # r = R * pts + t
                def rot(Ra, Rb, Rc, tcm):
                    m0 = tt(Ra, px, OP.mult)
                    m1 = tt(Rb, py, OP.mult)
                    a0 = tt(m0, m1, OP.add)
                    m2 = tt(Rc, pz, OP.mult)
                    a1 = tt(m2, tcm, OP.add)
                    return tt(a0, a1, OP.add)

                rx = rot(R00, R01, R02, tx)
                ry = rot(R10, R11, R12, ty)
                rz = rot(R20, R21, R22, tz)

                sqx_, sqy_, sqz_ = sq(rx), sq(ry), sq(rz)
                rho2 = tt(sqx_, sqy_, OP.add)
                r2 = tt(rho2, sqz_, OP.add)
                rng = T(); act.activation(rng[:], r2[:], AF.Sqrt)
                rho = T(); act.activation(rho[:], rho2[:], AF.Sqrt)

                Pq = tt(rx, rho2, OP.mult)
                ip = T()
                vec.reciprocal(ip[:], Pq[:])
                irx = tt(ip, rho2, OP.mult)
                irho2 = tt(ip, rx, OP.mult)
                irho = tt(rho, irho2, OP.mult)

                a1_ = tt(ry, irx, OP.mult)
                az0 = T(); act.activation(az0[:], a1_[:], AF.Arctan)
                sg = T(); act.activation(sg[:], ry[:], AF.Sign)
                msk = T()
                vec.tensor_scalar(out=msk[:], in0=rx[:], scalar1=0.0,
                                  scalar2=PI, op0=OP.is_lt, op1=OP.mult)
                corr = tt(msk, sg, OP.mult)
                az = tt(az0, corr, OP.add)

                e1 = tt(rz, irho, OP.mult)
                el = T(); act.activation(el[:], e1[:], AF.Arctan)

                for (pcomp, tgt, idx) in ((rng, X, 0), (az, Y, 1), (el, Z, 2)):
                    dsub = tt(pcomp, tgt, OP.subtract)
                    o = tt(dsub, W, OP.mult)
                    nc.sync.dma_start(out_d[idx, :, off:off + cc], o[:])
    nc.finalize()
    return nc


def _get(name, builder):
    if name not in _CACHE:
        _CACHE[name] = builder()
    return _CACHE[name]


def kernel(poses, patch_coords, elevation_angle, poses_idx, patch_idx,
           target_coords, weights):
    from concourse.bass_utils import run_bass_kernel_spmd

    poses = np.asarray(poses, dtype=np.float32)
    patch_coords = np.asarray(patch_coords, dtype=np.float32)
    elevation_angle = np.asarray(elevation_angle, dtype=np.float32)
    target_coords = np.asarray(target_coords, dtype=np.float32)
    weights = np.asarray(weights, dtype=np.float32)
    pid = np.asarray(poses_idx).astype(np.int64)
    qid = np.asarray(patch_idx).astype(np.int64)

    # ---- launch A: pose table -> rotation matrices (device) ----
    q_planes = np.ascontiguousarray(
        poses[:, 3:7].reshape(P, PC, 4).transpose(2, 0, 1))   # [4,128,32]
    ncA = _get("A", _build_posetab)
    resA = run_bass_kernel_spmd(ncA, [{"q": q_planes} for _ in range(NCORES)],
                                list(range(NCORES)))
    rtab = np.asarray(resA.results[0]["rtab"]).reshape(9, NPOSE).T  # [4096,9]

    # ---- host: gather derived tables / per-obs staging (indexing only) ----
    r9 = rtab[pid]                                            # [M, 9]
    t3 = poses[pid, 0:3]                                      # [M, 3]
    pts = np.concatenate(
        [patch_coords[qid], elevation_angle[qid]], axis=1)    # [M, 3]
    big = np.concatenate(
        [r9[:, 0:3], pts, t3[:, 0:1], r9[:, 3:6], t3[:, 1:2],
         r9[:, 6:9], t3[:, 2:3], target_coords, weights], axis=1)
    big = np.ascontiguousarray(
        big.reshape(NCORES, P, COLS, NPB).transpose(0, 3, 1, 2))

    # ---- launch B: streaming rotate+polar+residual ----
    ncB = _get("B", _build_main)
    resB = run_bass_kernel_spmd(ncB, [{"in": big[c]} for c in range(NCORES)],
                                list(range(NCORES)))
    out = np.stack([resB.results[c]["out"] for c in range(NCORES)])
    return np.ascontiguousarray(
        out.transpose(0, 2, 3, 1).reshape(M, 3)).astype(np.float32)



# revision 2
# speedup vs baseline: 1.4312x; 1.4312x over previous
"""Trainium2 Bass kernel for nn_BoundleAdjustment (2M observations).

Two launches on all 8 NeuronCores (observations data-parallel, M/8 per core):

Launch A (device): converts the 4096-row pose table (translation+quaternion)
into per-pose rotation matrices R = f(q/|q|) on the Vector engine
([128, 32] planar layout, one reciprocal for the 2/|q|^2 scale).

Host staging (indexing/layout only): gathers the derived R table, raw pose
translations, and patch rows by poses_idx/patch_idx, casts the per-
observation record planes to fp16, and lays them out as two contiguous
blocks per chunk so each chunk needs only two big DMAs.

Launch B (device): streams fp16 planes through SBUF in 2 chunks.
Rotation + residual math runs in fp16 on the Vector engine (2x DVE mode);
squares/sqrts/arctans on the Scalar engine; the azimuth uses the
half-angle identity az = 2*atan(ry/(rho+rx)) which needs no quadrant
fixup; the two reciprocals run in f32 via reciprocal_approx_fast with
max(x,1e-30) guards so no inf/NaN can form.
"""

import numpy as np

M = 2097152
NCORES = 8
N = M // NCORES
P = 128
COLS = N // P            # 2048
CC = 1024                # chunk cols
NCH = COLS // CC         # 2 chunks
NPOSE = 4096
PC = NPOSE // P          # 32 cols for pose table

# plane groups: A holds the rot-x chain (computed first), B the rest
NPA = 7                  # R00 R01 R02 px py pz tx
NPB_ = 12                # R10 R11 R12 ty R20 R21 R22 tz X Y Z W

_CACHE = {}


def _build_posetab():
    import concourse.tile as tile
    from concourse import bacc, mybir

    nc = bacc.Bacc("TRN2", target_bir_lowering=False, debug=False,
                   num_devices=NCORES)
    f32 = mybir.dt.float32
    OP = mybir.AluOpType
    q_d = nc.declare_dram_parameter("q", [4, P, PC], f32, isOutput=False)
    r_d = nc.declare_dram_parameter("rtab", [9, P, PC], f32, isOutput=True)

    with tile.TileContext(nc) as tc:
        with tc.tile_pool(name="pp", bufs=40) as pp:
            cnt = [0]

            def T():
                cnt[0] += 1
                return pp.tile([P, PC], f32, tag="t", name=f"pt{cnt[0]}")

            vec = nc.vector

            def tt(a, b, op):
                d = T()
                vec.tensor_tensor(out=d[:], in0=a[:], in1=b[:], op=op)
                return d

            qs = []
            for k in range(4):
                t = pp.tile([P, PC], f32, tag="t", name=f"qin{k}")
                nc.sync.dma_start(t[:], q_d[k])
                qs.append(t)
            qx, qy, qz, qw = qs

            xx = tt(qx, qx, OP.mult); yy = tt(qy, qy, OP.mult)
            zz = tt(qz, qz, OP.mult); ww = tt(qw, qw, OP.mult)
            xy = tt(qx, qy, OP.mult); xz = tt(qx, qz, OP.mult)
            yz = tt(qy, qz, OP.mult)
            wx = tt(qw, qx, OP.mult); wy = tt(qw, qy, OP.mult)
            wz = tt(qw, qz, OP.mult)

            s01 = tt(xx, yy, OP.add)
            s23 = tt(zz, ww, OP.add)
            s = tt(s01, s23, OP.add)
            d1 = T()
            vec.tensor_scalar(out=d1[:], in0=s[:], scalar1=0.5, scalar2=None,
                              op0=OP.mult)
            u = T()
            vec.reciprocal(u[:], d1[:])        # u = 2/|q|^2

            outs = {}

            def diag(m1, m2, nm):
                a = tt(m1, m2, OP.add)
                b = tt(a, u, OP.mult)
                d = T()
                vec.tensor_scalar(out=d[:], in0=b[:], scalar1=-1.0,
                                  scalar2=1.0, op0=OP.mult, op1=OP.add)
                outs[nm] = d

            def offd(m1, m2, op, nm):
                a = tt(m1, m2, op)
                outs[nm] = tt(a, u, OP.mult)

            diag(yy, zz, "R00"); diag(xx, zz, "R11"); diag(xx, yy, "R22")
            offd(xy, wz, OP.subtract, "R01"); offd(xz, wy, OP.add, "R02")
            offd(xy, wz, OP.add, "R10"); offd(yz, wx, OP.subtract, "R12")
            offd(xz, wy, OP.subtract, "R20"); offd(yz, wx, OP.add, "R21")

            for i, nm in enumerate(("R00", "R01", "R02", "R10", "R11", "R12",
                                    "R20", "R21", "R22")):
                nc.sync.dma_start(r_d[i], outs[nm][:])
    nc.finalize()
    return nc


def _build_main():
    import concourse.tile as tile
    from concourse import bacc, mybir

    nc = bacc.Bacc("TRN2", target_bir_lowering=False, debug=False,
                   num_devices=NCORES)
    f16 = mybir.dt.float16
    f32 = mybir.dt.float32
    AF = mybir.ActivationFunctionType
    OP = mybir.AluOpType
    inA_d = nc.declare_dram_parameter("inA", [NCH, P, NPA * CC], f16,
                                      isOutput=False)
    inB_d = nc.declare_dram_parameter("inB", [NCH, P, NPB_ * CC], f16,
                                      isOutput=False)
    out_d = nc.declare_dram_parameter("out", [NCH, P, 3 * CC], f16,
                                      isOutput=True)

    with tile.TileContext(nc) as tc:
        with tc.tile_pool(name="inp", bufs=2) as inp, \
             tc.tile_pool(name="outp", bufs=2) as outp, \
             tc.tile_pool(name="t16p", bufs=18) as t16p, \
             tc.tile_pool(name="t32p", bufs=10) as t32p:
            vec, act = nc.vector, nc.scalar
            for ch in range(NCH):
                tA = inp.tile([P, NPA * CC], f16, tag="inA", name=f"inA{ch}")
                nc.sync.dma_start(tA[:], inA_d[ch])
                tB = inp.tile([P, NPB_ * CC], f16, tag="inB", name=f"inB{ch}")
                nc.sync.dma_start(tB[:], inB_d[ch])
                ot = outp.tile([P, 3 * CC], f16, tag="out", name=f"out{ch}")

                vA = lambda k: tA[:, k * CC:(k + 1) * CC]
                vB = lambda k: tB[:, k * CC:(k + 1) * CC]
                R00, R01, R02, PX, PY, PZ, TX = (vA(k) for k in range(7))
                R10, R11, R12, TY = (vB(k) for k in range(4))
                R20, R21, R22, TZ = (vB(k) for k in range(4, 8))
                X, Y, Z, W = (vB(k) for k in range(8, 12))

                cnt = [0]

                def T16():
                    cnt[0] += 1
                    return t16p.tile([P, CC], f16, tag="t16",
                                     name=f"t16_{ch}_{cnt[0]}")

                def T32():
                    cnt[0] += 1
                    return t32p.tile([P, CC], f32, tag="t32",
                                     name=f"t32_{ch}_{cnt[0]}")

                def tt(a, b, op):
                    d = T16()
                    vec.tensor_tensor(out=d[:], in0=a, in1=b, op=op)
                    return d

                def ap(x):
                    return x[:] if hasattr(x, "tile_id") else x

                def row(Ra, Rb, Rc, Tc):
                    m0 = tt(Ra, PX, OP.mult)
                    m1 = tt(Rb, PY, OP.mult)
                    m2 = tt(Rc, PZ, OP.mult)
                    a0 = tt(m0[:], m1[:], OP.add)
                    a1 = tt(m2[:], Tc, OP.add)
                    return tt(a0[:], a1[:], OP.add)

                rx = row(R00, R01, R02, TX)
                ry = row(R10, R11, R12, TY)
                rz = row(R20, R21, R22, TZ)

                def sq(a):
                    d = T16()
                    act.activation(d[:], a[:], AF.Square)
                    return d

                sqx, sqy, sqz = sq(rx), sq(ry), sq(rz)
                rho2 = tt(sqx[:], sqy[:], OP.add)
                r2 = tt(rho2[:], sqz[:], OP.add)
                rng = T16(); act.activation(rng[:], r2[:], AF.Sqrt)
                rho = T32(); act.activation(rho[:], rho2[:], AF.Sqrt)

                rho_g = T32()
                vec.tensor_scalar(out=rho_g[:], in0=rho[:], scalar1=1e-30,
                                  scalar2=None, op0=OP.max)
                irho = T32(); vec.reciprocal_approx_fast(irho[:], rho_g[:])
                den = T32()
                vec.tensor_tensor(out=den[:], in0=rho_g[:], in1=rx[:],
                                  op=OP.add)
                den_g = T32()
                vec.tensor_scalar(out=den_g[:], in0=den[:], scalar1=1e-30,
                                  scalar2=None, op0=OP.max)
                iden = T32(); vec.reciprocal_approx_fast(iden[:], den_g[:])
                q = T32()
                vec.tensor_tensor(out=q[:], in0=ry[:], in1=iden[:], op=OP.mult)
                e1 = T32()
                vec.tensor_tensor(out=e1[:], in0=rz[:], in1=irho[:],
                                  op=OP.mult)
                az0 = T16(); act.activation(az0[:], q[:], AF.Arctan)
                el = T16(); act.activation(el[:], e1[:], AF.Arctan)

                dr = tt(rng[:], X, OP.subtract)
                da = T16()
                vec.scalar_tensor_tensor(out=da[:], in0=az0[:], scalar=2.0,
                                         in1=Y, op0=OP.mult, op1=OP.subtract)
                de = tt(el[:], Z, OP.subtract)
                vec.tensor_tensor(out=ot[:, 0:CC], in0=dr[:], in1=W,
                                  op=OP.mult)
                vec.tensor_tensor(out=ot[:, CC:2 * CC], in0=da[:], in1=W,
                                  op=OP.mult)
                vec.tensor_tensor(out=ot[:, 2 * CC:3 * CC], in0=de[:], in1=W,
                                  op=OP.mult)
                nc.sync.dma_start(out_d[ch], ot[:])
    nc.finalize()
    return nc


def _get(name, builder):
    if name not in _CACHE:
        _CACHE[name] = builder()
    return _CACHE[name]


def stage_q(poses):
    """[4,128,32] f32 quaternion planes for launch A."""
    return np.ascontiguousarray(
        poses[:, 3:7].reshape(P, PC, 4).transpose(2, 0, 1))


def stage_obs(rtab, poses, patch_coords, elevation_angle, pid, qid,
              target_coords, weights):
    """Gather per-observation planes, cast fp16, lay out per core/chunk.

    Returns (bigA [NCORES,NCH,P,NPA,CC] f16, bigB [NCORES,NCH,P,NPB_,CC] f16).
    """
    r9 = rtab[pid]                                            # [M, 9]
    t3 = poses[pid, 0:3]                                      # [M, 3]
    pts = np.concatenate(
        [patch_coords[qid], elevation_angle[qid]], axis=1)    # [M, 3]
    valA = np.concatenate([r9[:, 0:3], pts, t3[:, 0:1]], axis=1)
    valB = np.concatenate(
        [r9[:, 3:6], t3[:, 1:2], r9[:, 6:9], t3[:, 2:3],
         target_coords, weights], axis=1)

    def lay(v, np_):
        v = v.astype(np.float16)
        v = v.reshape(NCORES, P, NCH, CC, np_).transpose(0, 2, 1, 4, 3)
        return np.ascontiguousarray(v)

    return lay(valA, NPA), lay(valB, NPB_)


def unstage_out(res_list):
    """res_list: per-core [NCH,P,3,CC] f16 -> [M,3] f32."""
    out = np.stack([np.asarray(r).reshape(NCH, P, 3, CC) for r in res_list])
    out = out.transpose(0, 2, 1, 4, 3).reshape(M, 3)
    return np.ascontiguousarray(out).astype(np.float32)


def kernel(poses, patch_coords, elevation_angle, poses_idx, patch_idx,
           target_coords, weights):
    from concourse.bass_utils import run_bass_kernel_spmd

    poses = np.asarray(poses, dtype=np.float32)
    patch_coords = np.asarray(patch_coords, dtype=np.float32)
    elevation_angle = np.asarray(elevation_angle, dtype=np.float32)
    target_coords = np.asarray(target_coords, dtype=np.float32)
    weights = np.asarray(weights, dtype=np.float32)
    pid = np.asarray(poses_idx).astype(np.int64)
    qid = np.asarray(patch_idx).astype(np.int64)

    # ---- launch A: pose table -> rotation matrices (device) ----
    q_planes = stage_q(poses)
    ncA = _get("A", _build_posetab)
    resA = run_bass_kernel_spmd(ncA, [{"q": q_planes} for _ in range(NCORES)],
                                list(range(NCORES)))
    rtab = np.asarray(resA.results[0]["rtab"]).reshape(9, NPOSE).T  # [4096,9]

    # ---- host: gather + fp16 staging (indexing/layout only) ----
    bigA, bigB = stage_obs(rtab, poses, patch_coords, elevation_angle,
                           pid, qid, target_coords, weights)

    # ---- launch B: streaming rotate+polar+residual ----
    ncB = _get("B", _build_main)
    resB = run_bass_kernel_spmd(
        ncB, [{"inA": bigA[c], "inB": bigB[c]} for c in range(NCORES)],
        list(range(NCORES)))
    return unstage_out([resB.results[c]["out"] for c in range(NCORES)])


# revision 7
# speedup vs baseline: 1.5087x; 1.0542x over previous
"""Trainium2 Bass kernel for nn_BoundleAdjustment (2M observations).

Two launches on all 8 NeuronCores (observations data-parallel, M/8 per core):

Launch A (device): converts the 4096-row pose table (translation+quaternion)
into per-pose rotation matrices R = f(q/|q|) on the Vector engine
([128, 32] planar layout, one reciprocal for the 2/|q|^2 scale).

Host staging (indexing/layout only): gathers the derived R table, raw pose
translations, and patch rows by poses_idx/patch_idx, casts the per-
observation record planes to fp16, and lays them out as two contiguous
blocks per chunk so each chunk needs only two big DMAs.

Launch B (device): streams fp16 planes through SBUF in 2 chunks.
Rotation + residual math runs in fp16 on the Vector engine (2x DVE mode);
squares/sqrts/arctans on the Scalar engine; the azimuth uses the
half-angle identity az = 2*atan(ry/(rho+rx)) which needs no quadrant
fixup; the two reciprocals run in f32 via reciprocal_approx_fast with
max(x,1e-30) guards so no inf/NaN can form.
"""

import numpy as np

M = 2097152
NCORES = 8
N = M // NCORES
P = 128
COLS = N // P            # 2048
CC = 1024                # chunk cols
NCH = COLS // CC         # 2 chunks
NPOSE = 4096
PC = NPOSE // P          # 32 cols for pose table

# plane groups: A holds the rot-x chain (computed first), B the rest
NPA = 7                  # R00 R01 R02 px py pz tx
NPB_ = 12                # R10 R11 R12 ty R20 R21 R22 tz X Y Z W

_CACHE = {}


# launch A staged layout: 22 blocks of 32 cols, products prod_k = QA_k * QB_k
#   0-5   PL1 = yy xx xx xy xz yz      6-11  PL2 = zz zz yy wz wy wx
#   12-14 MN1 = xy xz yz               15-17 MN2 = wz wy wx
#   18-21 SS  = xx yy zz ww
# plus  = PL1+PL2 = [d00 d11 d22 o10 o02 o21], minus = MN1-MN2 = [o01 o20 o12]
_QA_IDX = [1, 0, 0, 0, 0, 1,  2, 2, 1, 3, 3, 3,  0, 0, 1,  3, 3, 3,  0, 1, 2, 3]
_QB_IDX = [1, 0, 0, 1, 2, 2,  2, 2, 1, 2, 1, 0,  1, 2, 2,  2, 1, 0,  0, 1, 2, 3]
NQB = 22


def _build_posetab():
    import concourse.tile as tile
    from concourse import bacc, mybir

    nc = bacc.Bacc("TRN2", target_bir_lowering=False, debug=False,
                   num_devices=NCORES)
    f32 = mybir.dt.float32
    OP = mybir.AluOpType
    qa_d = nc.declare_dram_parameter("qa", [P, NQB * PC], f32, isOutput=False)
    qb_d = nc.declare_dram_parameter("qb", [P, NQB * PC], f32, isOutput=False)
    r_d = nc.declare_dram_parameter("rtab", [P, 9 * PC], f32, isOutput=True)

    with tile.TileContext(nc) as tc:
        with tc.tile_pool(name="pp", bufs=12) as pp:
            vec = nc.vector
            qa = pp.tile([P, NQB * PC], f32, tag="qa", name="qa")
            nc.sync.dma_start(qa[:], qa_d[:, :])
            qb = pp.tile([P, NQB * PC], f32, tag="qb", name="qb")
            nc.sync.dma_start(qb[:], qb_d[:, :])
            rt = pp.tile([P, 9 * PC], f32, tag="rt", name="rt")

            def blk(t, i, n=1):
                return t[:, i * PC:(i + n) * PC]

            prod = pp.tile([P, NQB * PC], f32, tag="prod", name="prod")
            vec.tensor_tensor(out=prod[:], in0=qa[:], in1=qb[:], op=OP.mult)
            plus = pp.tile([P, 6 * PC], f32, tag="plus", name="plus")
            vec.tensor_tensor(out=plus[:], in0=blk(prod, 0, 6),
                              in1=blk(prod, 6, 6), op=OP.add)
            minus = pp.tile([P, 3 * PC], f32, tag="minus", name="minus")
            vec.tensor_tensor(out=minus[:], in0=blk(prod, 12, 3),
                              in1=blk(prod, 15, 3), op=OP.subtract)
            s2 = pp.tile([P, 2 * PC], f32, tag="s2", name="s2")
            vec.tensor_tensor(out=s2[:], in0=blk(prod, 18, 2),
                              in1=blk(prod, 20, 2), op=OP.add)
            d1 = pp.tile([P, PC], f32, tag="d1", name="d1")
            # d1 = 0.5*(xx+yy) + 0.5*(zz+ww) via STT: (a*0.5) + b*... do in 2
            vec.tensor_tensor(out=d1[:], in0=blk(s2, 0), in1=blk(s2, 1),
                              op=OP.add)
            dh = pp.tile([P, PC], f32, tag="dh", name="dh")
            vec.tensor_scalar(out=dh[:], in0=d1[:], scalar1=0.5, scalar2=None,
                              op0=OP.mult)
            u = pp.tile([P, PC], f32, tag="u", name="u")
            vec.reciprocal(u[:], dh[:])        # u = 2/|q|^2

            # off-diagonals: R order R00 R01 R02 R10 R11 R12 R20 R21 R22
            for src, dst in ((3, 3), (4, 2), (5, 7)):      # plus -> o10 o02 o21
                vec.tensor_tensor(out=blk(rt, dst), in0=blk(plus, src),
                                  in1=u[:], op=OP.mult)
            for src, dst in ((0, 1), (1, 6), (2, 5)):      # minus -> o01 o20 o12
                vec.tensor_tensor(out=blk(rt, dst), in0=blk(minus, src),
                                  in1=u[:], op=OP.mult)
            # diagonals: R_ii = 1 - u*(pair)
            dgm = pp.tile([P, 3 * PC], f32, tag="dgm", name="dgm")
            for i in range(3):
                vec.tensor_tensor(out=blk(dgm, i), in0=blk(plus, i),
                                  in1=u[:], op=OP.mult)
            for i, dst in enumerate((0, 4, 8)):
                vec.tensor_scalar(out=blk(rt, dst), in0=blk(dgm, i),
                                  scalar1=-1.0, scalar2=1.0,
                                  op0=OP.mult, op1=OP.add)
            nc.sync.dma_start(r_d[:, :], rt[:])
    nc.finalize()
    return nc


def _build_main():
    import concourse.tile as tile
    from concourse import bacc, mybir

    nc = bacc.Bacc("TRN2", target_bir_lowering=False, debug=False,
                   num_devices=NCORES)
    f16 = mybir.dt.float16
    f32 = mybir.dt.float32
    AF = mybir.ActivationFunctionType
    OP = mybir.AluOpType
    inA_d = nc.declare_dram_parameter("inA", [NCH, P, NPA * CC], f16,
                                      isOutput=False)
    inB_d = nc.declare_dram_parameter("inB", [NCH, P, NPB_ * CC], f16,
                                      isOutput=False)
    out_d = nc.declare_dram_parameter("out", [NCH, P, 3 * CC], f16,
                                      isOutput=True)

    with tile.TileContext(nc) as tc:
        with tc.tile_pool(name="inp", bufs=2) as inp, \
             tc.tile_pool(name="outp", bufs=2) as outp, \
             tc.tile_pool(name="t16p", bufs=30) as t16p, \
             tc.tile_pool(name="t32p", bufs=12) as t32p:
            vec, act = nc.vector, nc.scalar
            for ch in range(NCH):
                tA = inp.tile([P, NPA * CC], f16, tag="inA", name=f"inA{ch}")
                nc.sync.dma_start(tA[:], inA_d[ch])
                tB = inp.tile([P, NPB_ * CC], f16, tag="inB", name=f"inB{ch}")
                nc.sync.dma_start(tB[:], inB_d[ch])
                ot = outp.tile([P, 3 * CC], f16, tag="out", name=f"out{ch}")

                vA = lambda k: tA[:, k * CC:(k + 1) * CC]
                vB = lambda k: tB[:, k * CC:(k + 1) * CC]
                R00, R01, R02, PX, PY, PZ, TX = (vA(k) for k in range(7))
                R10, R11, R12, TY = (vB(k) for k in range(4))
                R20, R21, R22, TZ = (vB(k) for k in range(4, 8))
                X, Y, Z, W = (vB(k) for k in range(8, 12))

                cnt = [0]

                def T16():
                    cnt[0] += 1
                    return t16p.tile([P, CC], f16, tag="t16",
                                     name=f"t16_{ch}_{cnt[0]}")

                def T32():
                    cnt[0] += 1
                    return t32p.tile([P, CC], f32, tag="t32",
                                     name=f"t32_{ch}_{cnt[0]}")

                def tt(a, b, op):
                    d = T16()
                    vec.tensor_tensor(out=d[:], in0=a, in1=b, op=op)
                    return d

                def ap(x):
                    return x[:] if hasattr(x, "tile_id") else x

                def row(Ra, Rb, Rc, Tc):
                    m0 = tt(Ra, PX, OP.mult)
                    m1 = tt(Rb, PY, OP.mult)
                    m2 = tt(Rc, PZ, OP.mult)
                    a0 = tt(m0[:], m1[:], OP.add)
                    a1 = tt(m2[:], Tc, OP.add)
                    return tt(a0[:], a1[:], OP.add)

                rx = row(R00, R01, R02, TX)
                ry = row(R10, R11, R12, TY)
                rz = row(R20, R21, R22, TZ)

                def sq(a):
                    d = T16()
                    act.activation(d[:], a[:], AF.Square)
                    return d

                sqx, sqy, sqz = sq(rx), sq(ry), sq(rz)
                rho2 = tt(sqx[:], sqy[:], OP.add)
                r2 = tt(rho2[:], sqz[:], OP.add)
                rng = T16(); act.activation(rng[:], r2[:], AF.Sqrt)
                rho = T32(); act.activation(rho[:], rho2[:], AF.Sqrt)

                rho_g = T32()
                vec.tensor_scalar(out=rho_g[:], in0=rho[:], scalar1=1e-30,
                                  scalar2=None, op0=OP.max)
                irho = T32(); vec.reciprocal_approx_fast(irho[:], rho_g[:])
                den = T32()
                vec.tensor_tensor(out=den[:], in0=rho_g[:], in1=rx[:],
                                  op=OP.add)
                den_g = T32()
                vec.tensor_scalar(out=den_g[:], in0=den[:], scalar1=1e-30,
                                  scalar2=None, op0=OP.max)
                iden = T32(); vec.reciprocal_approx_fast(iden[:], den_g[:])
                q = T32()
                vec.tensor_tensor(out=q[:], in0=ry[:], in1=iden[:], op=OP.mult)
                e1 = T32()
                vec.tensor_tensor(out=e1[:], in0=rz[:], in1=irho[:],
                                  op=OP.mult)
                az0 = T16(); act.activation(az0[:], q[:], AF.Arctan)
                el = T16(); act.activation(el[:], e1[:], AF.Arctan)

                dr = tt(rng[:], X, OP.subtract)
                da = T16()
                vec.scalar_tensor_tensor(out=da[:], in0=az0[:], scalar=2.0,
                                         in1=Y, op0=OP.mult, op1=OP.subtract)
                de = tt(el[:], Z, OP.subtract)
                vec.tensor_tensor(out=ot[:, 0:CC], in0=dr[:], in1=W,
                                  op=OP.mult)
                vec.tensor_tensor(out=ot[:, CC:2 * CC], in0=da[:], in1=W,
                                  op=OP.mult)
                vec.tensor_tensor(out=ot[:, 2 * CC:3 * CC], in0=de[:], in1=W,
                                  op=OP.mult)
                nc.sync.dma_start(out_d[ch], ot[:])
    nc.finalize()
    return nc


def _get(name, builder):
    if name not in _CACHE:
        _CACHE[name] = builder()
    return _CACHE[name]


def stage_q(poses):
    """(qa, qb) [128, NQB*32] f32 operand planes for launch A's one big mult."""
    qp = poses[:, 3:7].reshape(P, PC, 4).transpose(2, 0, 1)  # [4,128,32]
    qa = np.concatenate([qp[i] for i in _QA_IDX], axis=1)
    qb = np.concatenate([qp[i] for i in _QB_IDX], axis=1)
    return np.ascontiguousarray(qa), np.ascontiguousarray(qb)


def decode_rtab(raw):
    """[128, 9*32] device layout -> [4096, 9] table."""
    r = np.asarray(raw).reshape(P, 9, PC).transpose(0, 2, 1)  # [128, 32, 9]
    return np.ascontiguousarray(r.reshape(NPOSE, 9))


def stage_obs(rtab, poses, patch_coords, elevation_angle, pid, qid,
              target_coords, weights):
    """Gather per-observation planes, cast fp16, lay out per core/chunk.

    Returns (bigA [NCORES,NCH,P,NPA,CC] f16, bigB [NCORES,NCH,P,NPB_,CC] f16).
    """
    r9 = rtab[pid]                                            # [M, 9]
    t3 = poses[pid, 0:3]                                      # [M, 3]
    pts = np.concatenate(
        [patch_coords[qid], elevation_angle[qid]], axis=1)    # [M, 3]
    valA = np.concatenate([r9[:, 0:3], pts, t3[:, 0:1]], axis=1)
    valB = np.concatenate(
        [r9[:, 3:6], t3[:, 1:2], r9[:, 6:9], t3[:, 2:3],
         target_coords, weights], axis=1)

    def lay(v, np_):
        v = v.astype(np.float16)
        v = v.reshape(NCORES, P, NCH, CC, np_).transpose(0, 2, 1, 4, 3)
        return np.ascontiguousarray(v)

    return lay(valA, NPA), lay(valB, NPB_)


def unstage_out(res_list):
    """res_list: per-core [NCH,P,3,CC] f16 -> [M,3] f32."""
    out = np.stack([np.asarray(r).reshape(NCH, P, 3, CC) for r in res_list])
    out = out.transpose(0, 2, 1, 4, 3).reshape(M, 3)
    return np.ascontiguousarray(out).astype(np.float32)


def kernel(poses, patch_coords, elevation_angle, poses_idx, patch_idx,
           target_coords, weights):
    from concourse.bass_utils import run_bass_kernel_spmd

    poses = np.asarray(poses, dtype=np.float32)
    patch_coords = np.asarray(patch_coords, dtype=np.float32)
    elevation_angle = np.asarray(elevation_angle, dtype=np.float32)
    target_coords = np.asarray(target_coords, dtype=np.float32)
    weights = np.asarray(weights, dtype=np.float32)
    pid = np.asarray(poses_idx).astype(np.int64)
    qid = np.asarray(patch_idx).astype(np.int64)

    # ---- launch A: pose table -> rotation matrices (device) ----
    qa, qb = stage_q(poses)
    ncA = _get("A", _build_posetab)
    resA = run_bass_kernel_spmd(ncA,
                                [{"qa": qa, "qb": qb} for _ in range(NCORES)],
                                list(range(NCORES)))
    rtab = decode_rtab(resA.results[0]["rtab"])

    # ---- host: gather + fp16 staging (indexing/layout only) ----
    bigA, bigB = stage_obs(rtab, poses, patch_coords, elevation_angle,
                           pid, qid, target_coords, weights)

    # ---- launch B: streaming rotate+polar+residual ----
    ncB = _get("B", _build_main)
    resB = run_bass_kernel_spmd(
        ncB, [{"inA": bigA[c], "inB": bigB[c]} for c in range(NCORES)],
        list(range(NCORES)))
    return unstage_out([resB.results[c]["out"] for c in range(NCORES)])
